# revision 27
# baseline (speedup 1.0000x reference)
"""Trainium2 Bass kernel for IrrepsLinear (128x0e + 128x1o + 128x2e).

y[n, off_l + o*d_l + d] = alpha * sum_m x[n, off_l + m*d_l + d] * W_l[m, o]

Data-parallel over nodes N across 8 cores; the whole data path runs in fp16
(fp32 accumulation in PSUM) — the harness gate is rel_err < 2e-2 and fp16
keeps it ~5e-4, while halving HBM traffic vs fp32 (the fp32 version sat at
the 358 GB/s HBM roofline).

Host-side sharding lays each core's x shard out m-major as
xg[128, 49, 9, 128] fp16: partition line m holds, for each 128-node subtile,
nine de-interleaved plane rows (one per (l, d) pair). A chunk of subtiles is
then a single contiguous DMA run per partition (up to ~16 KB descriptors).

On device the matmuls are W-stationary: the alpha-scaled weight (resident in
SBUF) is the stationary operand, x-planes stream as the moving operand, so
each subtile needs only 4 matmuls / 1344 PE cycles — the PE stays under the
DMA cadence even when HAM throttles it to half duty. Outputs land in PSUM
with partitions = o (weight out-channel): P1 [128, 2, 512] pairs l2 d0-3 for
two subtiles (one ACT copy per pair halves the per-instruction fixed cost),
P2 [128, 640] holds l1 d0-2 | l2 d4 | l0 (each matmul within one bank).
ACT/DVE copies cast fp32 -> fp16 into the plane-major SBUF tile, DMA'd to
the m-major output y[128, 49, (q, n)]; the host transposes o back against
nodes and inverse-permutes columns. Input DMAs ride the SP HWDGE ring,
output DMAs the ACT ring. Chunk sizes taper ([3,7,...,4,3,2,1,1]) so compute
starts early and the tail drains fast; 6 x/y buffers of prefetch depth ride
through HBM-contention bursts (the 8 cores pairwise share HBM stacks).
"""

import sys

sys.path.insert(0, "/opt/trn_rl_repo")

import numpy as np

N = 50000
FEAT = 1152
DIMS = [1, 3, 5]
OFFS = [0, 128, 512]
N_CORES = 8
SUB = 128            # nodes per subtile (partition dim)
NSUB = 49            # subtiles per core
NPC = NSUB * SUB     # padded nodes per core (6272)
SIZES = [3, 7, 7, 7, 7, 7, 7, 3, 1]   # subtiles per DMA unit (sum = 49)
CHMAX = max(SIZES)

# (l, d) plane order, both for the xg input and the plane-major output:
# P1 = l2 d0-3, then P2 = l1 d0-2 | l2 d4 | l0 (grouped so each W-stationary
# matmul streams a contiguous run of planes and stays within one PSUM bank).
PLANES = [(2, 0), (2, 1), (2, 2), (2, 3), (1, 0), (1, 1), (1, 2), (2, 4),
          (0, 0)]

_COMPILED = None


def build_nc(sizes=tuple(SIZES)):
    import concourse.mybir as mybir
    import concourse.tile as tile
    from concourse import bacc

    f16 = mybir.dt.float16
    f32 = mybir.dt.float32
    nsub = sum(sizes)

    nc = bacc.Bacc("TRN2", target_bir_lowering=False, debug=False,
                   num_devices=N_CORES)
    xg = nc.dram_tensor("xg", [128, nsub, 9, SUB], f16, kind="ExternalInput")
    w = nc.dram_tensor("w", [128, 3, 128], f16, kind="ExternalInput")
    y = nc.dram_tensor("y", [128, nsub, FEAT], f16, kind="ExternalOutput")

    with tile.TileContext(nc) as tc:
        with (
            tc.tile_pool(name="singles", bufs=1) as singles,
            tc.tile_pool(name="xs", bufs=6) as xpool,
            tc.tile_pool(name="ys", bufs=6) as ypool,
            tc.tile_pool(name="p1", bufs=2, space="PSUM") as p1pool,
            tc.tile_pool(name="p2", bufs=2, space="PSUM") as p2pool,
            # p1: 1 bank x 2 bufs; p2 pair tile: 3 banks x 2 bufs -> 8 banks
        ):
            # weights ride the ACT ring (idle this early) so chunk 0's input
            # DMA is the first thing issued on the SP ring
            wt = singles.tile([128, 3, 128], f16, tag="w")
            nc.scalar.dma_start(out=wt, in_=w[:, :, :])
            wts = [wt[:, i, :] for i in range(3)]

            s0 = 0
            for ci, csz in enumerate(sizes):
                xt = xpool.tile([128, CHMAX, 9, SUB], f16)
                nc.sync.dma_start(out=xt[:, 0:csz], in_=xg[:, s0:s0 + csz])
                yt = ypool.tile([128, CHMAX, FEAT], f16)
                for ai in range(0, csz, 2):
                    npair = min(2, csz - ai)

                    # W-stationary matmuls: weights are the stationary
                    # operand (lhsT), x-planes stream as the moving operand,
                    # so each subtile needs only 4 matmuls (1152 streamed
                    # columns) instead of 9 — keeps the PE under the DMA
                    # cadence even when HAM throttles it to half duty.
                    # Output partitions become o (weight out-channel); the
                    # host transposes o back against nodes.
                    # P1 pair: l2 d0-3 for two subtiles (two PSUM banks) —
                    # one scalar copy per pair halves the per-instruction
                    # fixed cost on the copy engines.
                    p1 = p1pool.tile([128, 2, 512], f32, tag="p1")
                    p2s = []
                    for j in range(npair):
                        # W2 planes grouped first to minimize weight reloads
                        nc.tensor.matmul(p1[:, j, :], lhsT=wts[2],
                                         rhs=xt[:, ai + j, 0:4, :])
                        # P2: l1 d0-2 | l2 d4 | l0 (two PSUM banks; every
                        # matmul stays within a single bank)
                        p2 = p2pool.tile([128, 640], f32, tag="p2")
                        nc.tensor.matmul(p2[:, 384:512], lhsT=wts[2],
                                         rhs=xt[:, ai + j, 7, :])
                        nc.tensor.matmul(p2[:, 0:384], lhsT=wts[1],
                                         rhs=xt[:, ai + j, 4:7, :])
                        nc.tensor.matmul(p2[:, 512:640], lhsT=wts[0],
                                         rhs=xt[:, ai + j, 8, :])
                        p2s.append(p2)

                    # PSUM -> SBUF copies (fp32 -> fp16 cast), plane-major
                    # output; host undoes the column permute.
                    nc.scalar.copy(yt[:, ai:ai + npair, 0:512],
                                   p1[:, 0:npair])
                    for j in range(npair):
                        nc.vector.tensor_copy(yt[:, ai + j, 512:1152],
                                              p2s[j])
                # output DMAs ride the ACT HWDGE ring (separate FIFO from
                # the input stream), except the final chunk: by then the SP
                # ring is drained, so issuing it there lets the last two
                # output transfers drain on both rings in parallel.
                eng = nc.sync if ci == len(sizes) - 1 else nc.scalar
                eng.dma_start(out=y[:, s0:s0 + csz], in_=yt[:, 0:csz])
                s0 += csz

    nc.compile()
    return nc


# plane q row m <- original feature column off_l + m*d_l + d; also the
# output-side permutation (plane-major column q*128+o -> natural column).
_PERM = np.concatenate([
    np.arange(128) * DIMS[l] + OFFS[l] + d for (l, d) in PLANES
])
_INV = np.empty(FEAT, np.int64)
_INV[_PERM] = np.arange(FEAT)


def _shard_inputs(x, W0, W1, W2):
    alpha = np.float32(1.0 / np.sqrt(128.0))
    ws = {"w": np.ascontiguousarray(
        np.stack([W0 * alpha, W1 * alpha, W2 * alpha], axis=1),
        dtype=np.float16)}
    x16 = np.asarray(x, dtype=np.float16)
    in_maps = []
    for i in range(N_CORES):
        lo = i * NPC
        hi = min(lo + NPC, N)
        xs = x16[lo:hi]
        xp = np.empty((9 * 128, NPC), np.float16)
        xp[:, : hi - lo] = xs.T[_PERM]
        if hi - lo < NPC:
            xp[:, hi - lo:] = 0.0
        # [9, 128m, nsub, 128n] -> m-major [128m, nsub, 9, 128n]
        xg = np.ascontiguousarray(
            xp.reshape(9, 128, NSUB, SUB).transpose(1, 2, 0, 3))
        in_maps.append({"xg": xg, **ws})
    return in_maps


def _unshard_output(results):
    out = np.empty((N, FEAT), np.float32)
    for i in range(N_CORES):
        lo = i * NPC
        hi = min(lo + NPC, N)
        # y[128o, nsub, (q,n)] -> node-major [(s,n), (q,o)]
        yp = results[i]["y"].reshape(128, NSUB, 9, SUB).transpose(
            1, 3, 2, 0).reshape(NPC, FEAT)[: hi - lo]
        out[lo:hi] = yp[:, _INV]
    return out


def _spot_check(out, x, Ws, rows):
    """Exact fp32 reference on a few rows; catches (rare) transient device
    corruption, which shows up at rel err ~0.2 vs the fp16 path's ~5e-4."""
    xs = np.asarray(x, np.float32)[rows]
    exp = np.empty((len(rows), FEAT), np.float32)
    for W, mul, dl, off in zip(Ws, [128, 128, 128], DIMS, OFFS):
        xl = xs[:, off:off + mul * dl].reshape(len(rows), mul, dl)
        alpha = np.float32(1.0 / np.sqrt(mul))
        yl = np.einsum("nmd,mo->nod", xl, np.asarray(W, np.float32)) * alpha
        exp[:, off:off + mul * dl] = yl.reshape(len(rows), mul * dl)
    rel = np.abs(out[rows] - exp).max() / max(np.abs(exp).max(), 1e-6)
    return rel


def kernel(x, W0, W1, W2):
    global _COMPILED
    from concourse.bass_utils import run_bass_kernel_spmd

    if _COMPILED is None:
        _COMPILED = build_nc()
    nc = _COMPILED
    in_maps = _shard_inputs(np.asarray(x), np.asarray(W0), np.asarray(W1),
                            np.asarray(W2))
    rows = np.random.default_rng(0).choice(N, 256, replace=False)
    out = None
    for attempt in range(3):
        try:
            res = run_bass_kernel_spmd(nc, in_maps, list(range(N_CORES)))
            out = _unshard_output(res.results)
        except Exception:
            if attempt == 2:
                raise
            continue
        if _spot_check(out, x, (W0, W1, W2), rows) < 5e-3:
            break
    return out


# revision 30
# speedup vs baseline: 1.0158x; 1.0158x over previous
"""Trainium2 Bass kernel for IrrepsLinear (128x0e + 128x1o + 128x2e).

y[n, off_l + o*d_l + d] = alpha * sum_m x[n, off_l + m*d_l + d] * W_l[m, o]

Data-parallel over nodes N across 8 cores; the whole data path runs in fp16
(fp32 accumulation in PSUM) — the harness gate is rel_err < 2e-2 and fp16
keeps it ~5e-4, while halving HBM traffic vs fp32 (the fp32 version sat at
the 358 GB/s HBM roofline).

Host-side sharding lays each core's x shard out m-major as
xg[128, 49, 9, 128] fp16: partition line m holds, for each 128-node subtile,
nine de-interleaved plane rows (one per (l, d) pair). A chunk of subtiles is
then a single contiguous DMA run per partition (up to ~16 KB descriptors).

On device the matmuls are W-stationary: the alpha-scaled weight (resident in
SBUF) is the stationary operand, x-planes stream as the moving operand, so
each subtile needs only 4 matmuls / 1344 PE cycles — the PE stays under the
DMA cadence even when HAM throttles it to half duty. Outputs land in PSUM
with partitions = o (weight out-channel): P1 [128, 2, 512] pairs l2 d0-3 for
two subtiles (one ACT copy per pair halves the per-instruction fixed cost),
P2 [128, 640] holds l1 d0-2 | l2 d4 | l0 (each matmul within one bank).
ACT/DVE copies cast fp32 -> fp16 into the plane-major SBUF tile, DMA'd to
the m-major output y[128, 49, (q, n)]; the host transposes o back against
nodes and inverse-permutes columns. Input DMAs ride the SP HWDGE ring,
output DMAs the ACT ring. Chunk sizes taper ([3,7,...,4,3,2,1,1]) so compute
starts early and the tail drains fast; 6 x/y buffers of prefetch depth ride
through HBM-contention bursts (the 8 cores pairwise share HBM stacks).
"""

import sys

sys.path.insert(0, "/opt/trn_rl_repo")

import numpy as np

N = 50000
FEAT = 1152
DIMS = [1, 3, 5]
OFFS = [0, 128, 512]
N_CORES = 8
SUB = 128            # nodes per subtile (partition dim)
NSUB = 49            # subtiles per core
NPC = NSUB * SUB     # padded nodes per core (6272)
SIZES = [3, 7, 7, 7, 7, 7, 7, 3, 1]   # subtiles per DMA unit (sum = 49)
CHMAX = max(SIZES)

# (l, d) plane order, both for the xg input and the plane-major output:
# P1 = l2 d0-3, then P2 = l1 d0-2 | l2 d4 | l0 (grouped so each W-stationary
# matmul streams a contiguous run of planes and stays within one PSUM bank).
PLANES = [(2, 0), (2, 1), (2, 2), (2, 3), (1, 0), (1, 1), (1, 2), (2, 4),
          (0, 0)]

_COMPILED = None


def build_nc(sizes=tuple(SIZES)):
    import concourse.mybir as mybir
    import concourse.tile as tile
    from concourse import bacc

    f16 = mybir.dt.float16
    f32 = mybir.dt.float32
    nsub = sum(sizes)

    nc = bacc.Bacc("TRN2", target_bir_lowering=False, debug=False,
                   num_devices=N_CORES)
    xg = nc.dram_tensor("xg", [128, nsub, 9, SUB], f16, kind="ExternalInput")
    w = nc.dram_tensor("w", [128, 3, 128], f16, kind="ExternalInput")
    y = nc.dram_tensor("y", [128, nsub, FEAT], f16, kind="ExternalOutput")

    with tile.TileContext(nc) as tc:
        with (
            tc.tile_pool(name="singles", bufs=1) as singles,
            tc.tile_pool(name="xs", bufs=6) as xpool,
            tc.tile_pool(name="ys", bufs=6) as ypool,
            tc.tile_pool(name="p1", bufs=2, space="PSUM") as p1pool,
            tc.tile_pool(name="p2", bufs=2, space="PSUM") as p2pool,
            # p1: 1 bank x 2 bufs; p2 pair tile: 3 banks x 2 bufs -> 8 banks
        ):
            # weights ride the ACT ring (idle this early) so chunk 0's input
            # DMA is the first thing issued on the SP ring
            wt = singles.tile([128, 3, 128], f16, tag="w")
            nc.scalar.dma_start(out=wt, in_=w[:, :, :])
            wts = [wt[:, i, :] for i in range(3)]

            s0 = 0
            for ci, csz in enumerate(sizes):
                xt = xpool.tile([128, CHMAX, 9, SUB], f16)
                nc.sync.dma_start(out=xt[:, 0:csz], in_=xg[:, s0:s0 + csz])
                yt = ypool.tile([128, CHMAX, FEAT], f16)
                # split large chunks' output into two DMAs, the first issued
                # mid-chunk so its bytes drain while the rest is computed
                half = 4 if csz > 4 else csz
                for ai in range(0, csz, 2):
                    npair = min(2, csz - ai)

                    # W-stationary matmuls: weights are the stationary
                    # operand (lhsT), x-planes stream as the moving operand,
                    # so each subtile needs only 4 matmuls (1152 streamed
                    # columns) instead of 9 — keeps the PE under the DMA
                    # cadence even when HAM throttles it to half duty.
                    # Output partitions become o (weight out-channel); the
                    # host transposes o back against nodes.
                    # P1 pair: l2 d0-3 for two subtiles (two PSUM banks) —
                    # one scalar copy per pair halves the per-instruction
                    # fixed cost on the copy engines.
                    p1 = p1pool.tile([128, 2, 512], f32, tag="p1")
                    p2s = []
                    for j in range(npair):
                        # W2 planes grouped first to minimize weight reloads
                        nc.tensor.matmul(p1[:, j, :], lhsT=wts[2],
                                         rhs=xt[:, ai + j, 0:4, :])
                        # P2: l1 d0-2 | l2 d4 | l0 (two PSUM banks; every
                        # matmul stays within a single bank)
                        p2 = p2pool.tile([128, 640], f32, tag="p2")
                        nc.tensor.matmul(p2[:, 384:512], lhsT=wts[2],
                                         rhs=xt[:, ai + j, 7, :])
                        nc.tensor.matmul(p2[:, 0:384], lhsT=wts[1],
                                         rhs=xt[:, ai + j, 4:7, :])
                        nc.tensor.matmul(p2[:, 512:640], lhsT=wts[0],
                                         rhs=xt[:, ai + j, 8, :])
                        p2s.append(p2)

                    # PSUM -> SBUF copies (fp32 -> fp16 cast), plane-major
                    # output; host undoes the column permute.
                    nc.scalar.copy(yt[:, ai:ai + npair, 0:512],
                                   p1[:, 0:npair])
                    for j in range(npair):
                        nc.vector.tensor_copy(yt[:, ai + j, 512:1152],
                                              p2s[j])
                    if ai + npair == half and half < csz:
                        nc.scalar.dma_start(out=y[:, s0:s0 + half],
                                            in_=yt[:, 0:half])
                # output DMAs ride the ACT HWDGE ring (separate FIFO from
                # the input stream), except the final chunk: by then the SP
                # ring is drained, so issuing it there lets the last two
                # output transfers drain on both rings in parallel.
                eng = nc.sync if ci == len(sizes) - 1 else nc.scalar
                lo = half if half < csz else 0
                eng.dma_start(out=y[:, s0 + lo:s0 + csz], in_=yt[:, lo:csz])
                s0 += csz

    nc.compile()
    return nc


# plane q row m <- original feature column off_l + m*d_l + d; also the
# output-side permutation (plane-major column q*128+o -> natural column).
_PERM = np.concatenate([
    np.arange(128) * DIMS[l] + OFFS[l] + d for (l, d) in PLANES
])
_INV = np.empty(FEAT, np.int64)
_INV[_PERM] = np.arange(FEAT)


def _shard_inputs(x, W0, W1, W2):
    alpha = np.float32(1.0 / np.sqrt(128.0))
    ws = {"w": np.ascontiguousarray(
        np.stack([W0 * alpha, W1 * alpha, W2 * alpha], axis=1),
        dtype=np.float16)}
    x16 = np.asarray(x, dtype=np.float16)
    in_maps = []
    for i in range(N_CORES):
        lo = i * NPC
        hi = min(lo + NPC, N)
        xs = x16[lo:hi]
        xp = np.empty((9 * 128, NPC), np.float16)
        xp[:, : hi - lo] = xs.T[_PERM]
        if hi - lo < NPC:
            xp[:, hi - lo:] = 0.0
        # [9, 128m, nsub, 128n] -> m-major [128m, nsub, 9, 128n]
        xg = np.ascontiguousarray(
            xp.reshape(9, 128, NSUB, SUB).transpose(1, 2, 0, 3))
        in_maps.append({"xg": xg, **ws})
    return in_maps


def _unshard_output(results):
    out = np.empty((N, FEAT), np.float32)
    for i in range(N_CORES):
        lo = i * NPC
        hi = min(lo + NPC, N)
        # y[128o, nsub, (q,n)] -> node-major [(s,n), (q,o)]
        yp = results[i]["y"].reshape(128, NSUB, 9, SUB).transpose(
            1, 3, 2, 0).reshape(NPC, FEAT)[: hi - lo]
        out[lo:hi] = yp[:, _INV]
    return out


def _spot_check(out, x, Ws, rows):
    """Exact fp32 reference on a few rows; catches (rare) transient device
    corruption, which shows up at rel err ~0.2 vs the fp16 path's ~5e-4."""
    xs = np.asarray(x, np.float32)[rows]
    exp = np.empty((len(rows), FEAT), np.float32)
    for W, mul, dl, off in zip(Ws, [128, 128, 128], DIMS, OFFS):
        xl = xs[:, off:off + mul * dl].reshape(len(rows), mul, dl)
        alpha = np.float32(1.0 / np.sqrt(mul))
        yl = np.einsum("nmd,mo->nod", xl, np.asarray(W, np.float32)) * alpha
        exp[:, off:off + mul * dl] = yl.reshape(len(rows), mul * dl)
    rel = np.abs(out[rows] - exp).max() / max(np.abs(exp).max(), 1e-6)
    return rel


def kernel(x, W0, W1, W2):
    global _COMPILED
    from concourse.bass_utils import run_bass_kernel_spmd

    if _COMPILED is None:
        _COMPILED = build_nc()
    nc = _COMPILED
    in_maps = _shard_inputs(np.asarray(x), np.asarray(W0), np.asarray(W1),
                            np.asarray(W2))
    rows = np.random.default_rng(0).choice(N, 256, replace=False)
    out = None
    for attempt in range(3):
        try:
            res = run_bass_kernel_spmd(nc, in_maps, list(range(N_CORES)))
            out = _unshard_output(res.results)
        except Exception:
            if attempt == 2:
                raise
            continue
        if _spot_check(out, x, (W0, W1, W2), rows) < 5e-3:
            break
    return out


# revision 31
# speedup vs baseline: 1.1453x; 1.1276x over previous
"""Trainium2 Bass kernel for IrrepsLinear (128x0e + 128x1o + 128x2e).

y[n, off_l + o*d_l + d] = alpha * sum_m x[n, off_l + m*d_l + d] * W_l[m, o]

Data-parallel over nodes N across 8 cores; the whole data path runs in fp16
(fp32 accumulation in PSUM) — the harness gate is rel_err < 2e-2 and fp16
keeps it ~5e-4, while halving HBM traffic vs fp32 (the fp32 version sat at
the 358 GB/s HBM roofline).

Host-side sharding lays each core's x shard out m-major as
xg[128, 49, 9, 128] fp16: partition line m holds, for each 128-node subtile,
nine de-interleaved plane rows (one per (l, d) pair). A chunk of subtiles is
then a single contiguous DMA run per partition (up to ~16 KB descriptors).

On device the matmuls are W-stationary: the alpha-scaled weight (resident in
SBUF) is the stationary operand, x-planes stream as the moving operand, so
each subtile needs only 4 matmuls / 1344 PE cycles — the PE stays under the
DMA cadence even when HAM throttles it to half duty. Outputs land in PSUM
with partitions = o (weight out-channel): P1 [128, 2, 512] pairs l2 d0-3 for
two subtiles (one ACT copy per pair halves the per-instruction fixed cost),
P2 [128, 640] holds l1 d0-2 | l2 d4 | l0 (each matmul within one bank).
ACT/DVE copies cast fp32 -> fp16 into the plane-major SBUF tile, DMA'd to
the m-major output y[128, 49, (q, n)]; the host transposes o back against
nodes and inverse-permutes columns. Input DMAs ride the SP HWDGE ring,
output DMAs the ACT ring. Chunk sizes taper ([3,7,...,4,3,2,1,1]) so compute
starts early and the tail drains fast; 6 x/y buffers of prefetch depth ride
through HBM-contention bursts (the 8 cores pairwise share HBM stacks).
"""

import sys

sys.path.insert(0, "/opt/trn_rl_repo")

import numpy as np

N = 50000
FEAT = 1152
DIMS = [1, 3, 5]
OFFS = [0, 128, 512]
N_CORES = 8
SUB = 128            # nodes per subtile (partition dim)
NSUB = 49            # subtiles per core
NPC = NSUB * SUB     # padded nodes per core (6272)
SIZES = [3, 7, 7, 7, 7, 7, 7, 3, 1]   # subtiles per DMA unit (sum = 49)
CHMAX = max(SIZES)

# (l, d) plane order, both for the xg input and the plane-major output:
# P1 = l2 d0-3, then P2 = l1 d0-2 | l2 d4 | l0 (grouped so each W-stationary
# matmul streams a contiguous run of planes and stays within one PSUM bank).
PLANES = [(2, 0), (2, 1), (2, 2), (2, 3), (1, 0), (1, 1), (1, 2), (2, 4),
          (0, 0)]

_COMPILED = None


def build_nc(sizes=tuple(SIZES)):
    import concourse.mybir as mybir
    import concourse.tile as tile
    from concourse import bacc

    f16 = mybir.dt.float16
    f32 = mybir.dt.float32
    nsub = sum(sizes)

    nc = bacc.Bacc("TRN2", target_bir_lowering=False, debug=False,
                   num_devices=N_CORES)
    xg = nc.dram_tensor("xg", [128, nsub, 9, SUB], f16, kind="ExternalInput")
    w = nc.dram_tensor("w", [128, 3, 128], f16, kind="ExternalInput")
    y = nc.dram_tensor("y", [128, nsub, FEAT], f16, kind="ExternalOutput")

    with tile.TileContext(nc) as tc:
        with (
            tc.tile_pool(name="singles", bufs=1) as singles,
            tc.tile_pool(name="xs", bufs=6) as xpool,
            tc.tile_pool(name="ys", bufs=6) as ypool,
            tc.tile_pool(name="p1", bufs=2, space="PSUM") as p1pool,
            tc.tile_pool(name="p2", bufs=2, space="PSUM") as p2pool,
            # p1: 1 bank x 2 bufs; p2 pair tile: 3 banks x 2 bufs -> 8 banks
        ):
            # weights ride the ACT ring (idle this early) so chunk 0's input
            # DMA is the first thing issued on the SP ring
            wt = singles.tile([128, 3, 128], f16, tag="w")
            nc.scalar.dma_start(out=wt, in_=w[:, :, :])
            wts = [wt[:, i, :] for i in range(3)]

            s0 = 0
            for ci, csz in enumerate(sizes):
                xt = xpool.tile([128, CHMAX, 9, SUB], f16)
                # split the input DMA so matmuls on the first subtiles start
                # as soon as their half lands instead of waiting for the
                # whole chunk (chunk 0 splits at 1 so compute begins with
                # the very first subtile)
                xcut = 1 if ci == 0 else (4 if csz > 4 else csz)
                nc.sync.dma_start(out=xt[:, 0:xcut], in_=xg[:, s0:s0 + xcut])
                if xcut < csz:
                    nc.sync.dma_start(out=xt[:, xcut:csz],
                                      in_=xg[:, s0 + xcut:s0 + csz])
                yt = ypool.tile([128, CHMAX, FEAT], f16)
                # split large chunks' output into two DMAs, the first issued
                # mid-chunk so its bytes drain while the rest is computed
                half = 4 if csz > 4 else csz
                for ai in range(0, csz, 2):
                    npair = min(2, csz - ai)

                    # W-stationary matmuls: weights are the stationary
                    # operand (lhsT), x-planes stream as the moving operand,
                    # so each subtile needs only 4 matmuls (1152 streamed
                    # columns) instead of 9 — keeps the PE under the DMA
                    # cadence even when HAM throttles it to half duty.
                    # Output partitions become o (weight out-channel); the
                    # host transposes o back against nodes.
                    # P1 pair: l2 d0-3 for two subtiles (two PSUM banks) —
                    # one scalar copy per pair halves the per-instruction
                    # fixed cost on the copy engines.
                    p1 = p1pool.tile([128, 2, 512], f32, tag="p1")
                    p2s = []
                    for j in range(npair):
                        # W2 planes grouped first to minimize weight reloads
                        nc.tensor.matmul(p1[:, j, :], lhsT=wts[2],
                                         rhs=xt[:, ai + j, 0:4, :])
                        # P2: l1 d0-2 | l2 d4 | l0 (two PSUM banks; every
                        # matmul stays within a single bank)
                        p2 = p2pool.tile([128, 640], f32, tag="p2")
                        nc.tensor.matmul(p2[:, 384:512], lhsT=wts[2],
                                         rhs=xt[:, ai + j, 7, :])
                        nc.tensor.matmul(p2[:, 0:384], lhsT=wts[1],
                                         rhs=xt[:, ai + j, 4:7, :])
                        nc.tensor.matmul(p2[:, 512:640], lhsT=wts[0],
                                         rhs=xt[:, ai + j, 8, :])
                        p2s.append(p2)

                    # PSUM -> SBUF copies (fp32 -> fp16 cast), plane-major
                    # output; host undoes the column permute.
                    nc.scalar.copy(yt[:, ai:ai + npair, 0:512],
                                   p1[:, 0:npair])
                    for j in range(npair):
                        nc.vector.tensor_copy(yt[:, ai + j, 512:1152],
                                              p2s[j])
                    if ai + npair == half and half < csz:
                        nc.scalar.dma_start(out=y[:, s0:s0 + half],
                                            in_=yt[:, 0:half])
                # output DMAs ride the ACT HWDGE ring (separate FIFO from
                # the input stream), except the final chunk: by then the SP
                # ring is drained, so issuing it there lets the last two
                # output transfers drain on both rings in parallel.
                eng = nc.sync if ci == len(sizes) - 1 else nc.scalar
                lo = half if half < csz else 0
                eng.dma_start(out=y[:, s0 + lo:s0 + csz], in_=yt[:, lo:csz])
                s0 += csz

    nc.compile()
    return nc


# plane q row m <- original feature column off_l + m*d_l + d; also the
# output-side permutation (plane-major column q*128+o -> natural column).
_PERM = np.concatenate([
    np.arange(128) * DIMS[l] + OFFS[l] + d for (l, d) in PLANES
])
_INV = np.empty(FEAT, np.int64)
_INV[_PERM] = np.arange(FEAT)


def _shard_inputs(x, W0, W1, W2):
    alpha = np.float32(1.0 / np.sqrt(128.0))
    ws = {"w": np.ascontiguousarray(
        np.stack([W0 * alpha, W1 * alpha, W2 * alpha], axis=1),
        dtype=np.float16)}
    x16 = np.asarray(x, dtype=np.float16)
    in_maps = []
    for i in range(N_CORES):
        lo = i * NPC
        hi = min(lo + NPC, N)
        xs = x16[lo:hi]
        xp = np.empty((9 * 128, NPC), np.float16)
        xp[:, : hi - lo] = xs.T[_PERM]
        if hi - lo < NPC:
            xp[:, hi - lo:] = 0.0
        # [9, 128m, nsub, 128n] -> m-major [128m, nsub, 9, 128n]
        xg = np.ascontiguousarray(
            xp.reshape(9, 128, NSUB, SUB).transpose(1, 2, 0, 3))
        in_maps.append({"xg": xg, **ws})
    return in_maps


def _unshard_output(results):
    out = np.empty((N, FEAT), np.float32)
    for i in range(N_CORES):
        lo = i * NPC
        hi = min(lo + NPC, N)
        # y[128o, nsub, (q,n)] -> node-major [(s,n), (q,o)]
        yp = results[i]["y"].reshape(128, NSUB, 9, SUB).transpose(
            1, 3, 2, 0).reshape(NPC, FEAT)[: hi - lo]
        out[lo:hi] = yp[:, _INV]
    return out


def _spot_check(out, x, Ws, rows):
    """Exact fp32 reference on a few rows; catches (rare) transient device
    corruption, which shows up at rel err ~0.2 vs the fp16 path's ~5e-4."""
    xs = np.asarray(x, np.float32)[rows]
    exp = np.empty((len(rows), FEAT), np.float32)
    for W, mul, dl, off in zip(Ws, [128, 128, 128], DIMS, OFFS):
        xl = xs[:, off:off + mul * dl].reshape(len(rows), mul, dl)
        alpha = np.float32(1.0 / np.sqrt(mul))
        yl = np.einsum("nmd,mo->nod", xl, np.asarray(W, np.float32)) * alpha
        exp[:, off:off + mul * dl] = yl.reshape(len(rows), mul * dl)
    rel = np.abs(out[rows] - exp).max() / max(np.abs(exp).max(), 1e-6)
    return rel


def kernel(x, W0, W1, W2):
    global _COMPILED
    from concourse.bass_utils import run_bass_kernel_spmd

    if _COMPILED is None:
        _COMPILED = build_nc()
    nc = _COMPILED
    in_maps = _shard_inputs(np.asarray(x), np.asarray(W0), np.asarray(W1),
                            np.asarray(W2))
    rows = np.random.default_rng(0).choice(N, 256, replace=False)
    out = None
    for attempt in range(3):
        try:
            res = run_bass_kernel_spmd(nc, in_maps, list(range(N_CORES)))
            out = _unshard_output(res.results)
        except Exception:
            if attempt == 2:
                raise
            continue
        if _spot_check(out, x, (W0, W1, W2), rows) < 5e-3:
            break
    return out
